# revision 2
# baseline (speedup 1.0000x reference)
"""Trainium2 Bass kernel for nn_ConvNetLayer (GNN message passing layer).

Strategy (8 NeuronCores, SPMD):
  - Sort edges by src atom; partition atoms into 8 contiguous ranges of 12500
    (graph-parallel). Each core owns its atom range and the edges whose src
    falls in it, so the segment_sum is core-local (no collectives).
  - Within a core, atoms are processed in 128-atom blocks; each block's edge
    list is padded to a uniform E_blk so all 8 cores run one identical program.
  - Per edge subtile (128 edges): A = bond @ Aw.T via PE (fp32r), C/V GEMMs
    computed in edge space from x[dst] rows gathered by indirect DMA,
    B x[src] term applied via a one-hot expand matmul from an SBUF-resident
    per-block Bx table (biases folded), all accumulated in PSUM.
    relu/sigmoid on ACT, gating mul on DVE, segment-sum via one-hot matmul
    accumulated per atom block in PSUM, finalized with the U GEMM + relu.
  - Host un-permutes bond_layer_output rows and assembles atom_layer_output.
"""

import numpy as np

import concourse.bacc as bacc
import concourse.mybir as mybir
import concourse.tile as tile
from concourse import bass
from concourse.bass_utils import run_bass_kernel_spmd

NCORES = 8
P = 128
H = 256

f32 = mybir.dt.float32
f32r = mybir.dt.float32r
i32 = mybir.dt.int32

_PROG_CACHE = {}


def _build_program(NB, S, V):
    """NB: atom blocks per core, S: subtiles per block, V: full atom rows."""
    nc = bacc.Bacc("TRN2", target_bir_lowering=False, debug=False,
                   num_devices=NCORES)
    A_pc = NB * P
    E_pc = NB * S * P

    bond_e = nc.dram_tensor("bond", [E_pc, H], f32, kind="ExternalInput")
    xfull_e = nc.dram_tensor("xfull", [V, H], f32, kind="ExternalInput")
    atom_e = nc.dram_tensor("atom", [A_pc, H], f32, kind="ExternalInput")
    dsti_e = nc.dram_tensor("dsti", [NB, P, S], i32, kind="ExternalInput")
    srcc_e = nc.dram_tensor("srcc", [NB, P, S], f32, kind="ExternalInput")
    srcr_e = nc.dram_tensor("srcr", [NB, S * P], f32, kind="ExternalInput")
    wt_e = {}
    for w in ("AwT", "BwT", "CwT", "VwT", "UwT"):
        wt_e[w] = nc.dram_tensor(w, [H, H], f32, kind="ExternalInput")
    bb_e = nc.dram_tensor("bb", [1, H], f32, kind="ExternalInput")
    vb_e = nc.dram_tensor("vb", [1, H], f32, kind="ExternalInput")
    ub_e = nc.dram_tensor("ub", [1, H], f32, kind="ExternalInput")
    iott_e = nc.dram_tensor("iott", [P, P], f32, kind="ExternalInput")
    iotc_e = nc.dram_tensor("iotc", [P, 1], f32, kind="ExternalInput")
    ones_e = nc.dram_tensor("ones", [1, P], f32, kind="ExternalInput")
    ident_e = nc.dram_tensor("ident", [P, P], f32, kind="ExternalInput")

    bondout_e = nc.dram_tensor("bondout", [E_pc, H], f32, kind="ExternalOutput")
    atomout_e = nc.dram_tensor("atomout", [A_pc, H], f32, kind="ExternalOutput")

    with tile.TileContext(nc) as tc:
        with (
            tc.tile_pool(name="const", bufs=1) as const,
            tc.tile_pool(name="bxa", bufs=1) as bxa_pool,
            tc.tile_pool(name="io", bufs=3) as io,
            tc.tile_pool(name="work", bufs=3) as work,
            tc.tile_pool(name="pgv", bufs=2, space="PSUM") as pgv,
            tc.tile_pool(name="ptp", bufs=2, space="PSUM") as ptp,
            tc.tile_pool(name="pbc", bufs=1, space="PSUM") as pbc,
            tc.tile_pool(name="pagg", bufs=2, space="PSUM") as pagg,
        ):
            # constants
            wt = {}
            for w in ("AwT", "BwT", "CwT", "VwT", "UwT"):
                wtile = const.tile([P, 2, H], f32r, name=f"w_{w}")
                nc.sync.dma_start(
                    out=wtile[:],
                    in_=wt_e[w][:].rearrange("(k p) h -> p k h", p=P).bitcast(f32r),
                )
                wt[w] = wtile
            bb_t = const.tile([1, H], f32r)
            vb_t = const.tile([1, H], f32r)
            ub_t = const.tile([1, H], f32r)
            for t, e in ((bb_t, bb_e), (vb_t, vb_e), (ub_t, ub_e)):
                nc.sync.dma_start(out=t[:], in_=e[:].bitcast(f32r))
            iott_t = const.tile([P, P], f32)
            nc.sync.dma_start(out=iott_t[:], in_=iott_e[:])
            iotc_t = const.tile([P, 1], f32)
            nc.sync.dma_start(out=iotc_t[:], in_=iotc_e[:])
            ones_t = const.tile([1, P], f32r)
            nc.sync.dma_start(out=ones_t[:], in_=ones_e[:].bitcast(f32r))
            ident_t = const.tile([P, P], f32r)
            nc.sync.dma_start(out=ident_t[:], in_=ident_e[:].bitcast(f32r))

            Bxa = bxa_pool.tile([P, NB, H], f32r)

            def transpose_pair(src_tile, name):
                """[128, 256] f32r -> two [128,128] transposed chunks in SBUF."""
                tT = work.tile([P, 2, P], f32r, name=name, tag=name)
                tp = ptp.tile([P, 2, P], f32r, space="PSUM", tag="tp")
                for k in range(2):
                    nc.tensor.transpose(
                        out=tp[:, k, :], in_=src_tile[:, k * P:(k + 1) * P],
                        identity=ident_t[:],
                    )
                eng = (nc.vector, nc.scalar)
                for k in range(2):
                    nc.vector.tensor_copy(out=tT[:, k, :], in_=tp[:, k, :])
                return tT

            # ---- phase 1: Bx table for own atoms (combined bias folded) ----
            for b in range(NB):
                atom_t = io.tile([P, H], f32r, name="atom_t", tag="atom_t")
                nc.sync.dma_start(
                    out=atom_t[:], in_=atom_e[b * P:(b + 1) * P, :].bitcast(f32r)
                )
                atT = transpose_pair(atom_t, "atT")
                bx_ps = pgv.tile([P, H], f32, space="PSUM", tag="gv")
                nc.tensor.matmul(out=bx_ps[:], lhsT=atT[:, 0, :],
                                 rhs=wt["BwT"][:, 0, :], start=True, stop=False)
                nc.tensor.matmul(out=bx_ps[:], lhsT=atT[:, 1, :],
                                 rhs=wt["BwT"][:, 1, :], start=False, stop=False)
                nc.tensor.matmul(out=bx_ps[:], lhsT=ones_t[:1, :], rhs=bb_t[:1, :],
                                 start=False, stop=True)
                nc.vector.tensor_copy(out=Bxa[:, b, :], in_=bx_ps[:])

            # ---- phase 2: edge blocks ----
            for b in range(NB):
                dsti_t = io.tile([P, S], i32, name="dsti_t", tag="dsti_t")
                nc.sync.dma_start(out=dsti_t[:], in_=dsti_e[b])
                srcc_t = io.tile([P, S], f32, name="srcc_t", tag="srcc_t")
                nc.sync.dma_start(out=srcc_t[:], in_=srcc_e[b])
                srcr_t = io.tile([1, S * P], f32r, name="srcr_t", tag="srcr_t")
                nc.sync.dma_start(out=srcr_t[:], in_=srcr_e[b, None, :].bitcast(f32r))
                bond_t = io.tile([P, S, H], f32r, name="bond_t", tag="bond_t")
                nc.sync.dma_start(
                    out=bond_t[:],
                    in_=bond_e[b * S * P:(b + 1) * S * P, :]
                    .rearrange("(j p) h -> p j h", p=P).bitcast(f32r),
                )
                bondout_t = io.tile([P, S, H], f32, name="bondout_t", tag="bondout_t")
                agg_ps = pagg.tile([P, H], f32, space="PSUM", tag="agg")

                for j in range(S):
                    xd_t = work.tile([P, H], f32r, name="xd_t", tag="xd_t")
                    nc.gpsimd.indirect_dma_start(
                        out=xd_t[:],
                        out_offset=None,
                        in_=xfull_e[:].bitcast(f32r),
                        in_offset=bass.IndirectOffsetOnAxis(
                            ap=dsti_t[:, j:j + 1], axis=0),
                    )
                    bT = transpose_pair(bond_t[:, j, :], "bT")
                    xdT = transpose_pair(xd_t, "xdT")

                    # one-hots
                    oh_em = work.tile([P, P], f32r, name="oh_em", tag="oh_em")
                    nc.vector.tensor_scalar(
                        out=oh_em[:], in0=iott_t[:], scalar1=srcc_t[:, j:j + 1],
                        scalar2=None, op0=mybir.AluOpType.is_equal,
                    )
                    srcbc_ps = pbc.tile([P, P], f32, space="PSUM", tag="bc")
                    nc.tensor.matmul(
                        out=srcbc_ps[:], lhsT=ones_t[:1, :],
                        rhs=srcr_t[:1, j * P:(j + 1) * P], start=True, stop=True,
                    )
                    oh_am = work.tile([P, P], f32r, name="oh_am", tag="oh_am")
                    nc.vector.tensor_scalar(
                        out=oh_am[:], in0=srcbc_ps[:], scalar1=iotc_t[:, :1],
                        scalar2=None, op0=mybir.AluOpType.is_equal,
                    )

                    # g and v accumulation (shared PSUM bank pair)
                    gv_ps = pgv.tile([P, 2 * H], f32, space="PSUM", tag="gv")
                    g = gv_ps[:, :H]
                    v = gv_ps[:, H:]
                    nc.tensor.matmul(out=g, lhsT=bT[:, 0, :], rhs=wt["AwT"][:, 0, :],
                                     start=True, stop=False)
                    nc.tensor.matmul(out=g, lhsT=bT[:, 1, :], rhs=wt["AwT"][:, 1, :],
                                     start=False, stop=False)
                    nc.tensor.matmul(out=g, lhsT=xdT[:, 0, :], rhs=wt["CwT"][:, 0, :],
                                     start=False, stop=False)
                    nc.tensor.matmul(out=g, lhsT=xdT[:, 1, :], rhs=wt["CwT"][:, 1, :],
                                     start=False, stop=False)
                    nc.tensor.matmul(out=g, lhsT=oh_am[:], rhs=Bxa[:, b, :],
                                     start=False, stop=True)
                    nc.tensor.matmul(out=v, lhsT=xdT[:, 0, :], rhs=wt["VwT"][:, 0, :],
                                     start=True, stop=False)
                    nc.tensor.matmul(out=v, lhsT=xdT[:, 1, :], rhs=wt["VwT"][:, 1, :],
                                     start=False, stop=False)
                    nc.tensor.matmul(out=v, lhsT=ones_t[:1, :], rhs=vb_t[:1, :],
                                     start=False, stop=True)

                    nc.scalar.activation(
                        out=bondout_t[:, j, :], in_=g,
                        func=mybir.ActivationFunctionType.Relu,
                    )
                    sig_t = work.tile([P, H], f32, name="sig_t", tag="sig_t")
                    nc.scalar.activation(
                        out=sig_t[:], in_=g,
                        func=mybir.ActivationFunctionType.Sigmoid,
                    )
                    msgs_t = work.tile([P, H], f32r, name="msgs_t", tag="msgs_t")
                    nc.vector.tensor_tensor(
                        out=msgs_t[:], in0=sig_t[:], in1=v,
                        op=mybir.AluOpType.mult,
                    )
                    nc.tensor.matmul(out=agg_ps[:], lhsT=oh_em[:], rhs=msgs_t[:],
                                     start=(j == 0), stop=False)

                nc.sync.dma_start(
                    out=bondout_e[b * S * P:(b + 1) * S * P, :]
                    .rearrange("(j p) h -> p j h", p=P),
                    in_=bondout_t[:],
                )

                # finalize block: U GEMM + Ub + relu
                atomu_t = io.tile([P, H], f32r, name="atomu_t", tag="atomu_t")
                nc.sync.dma_start(
                    out=atomu_t[:], in_=atom_e[b * P:(b + 1) * P, :].bitcast(f32r)
                )
                auT = transpose_pair(atomu_t, "auT")
                nc.tensor.matmul(out=agg_ps[:], lhsT=auT[:, 0, :],
                                 rhs=wt["UwT"][:, 0, :], start=False, stop=False)
                nc.tensor.matmul(out=agg_ps[:], lhsT=auT[:, 1, :],
                                 rhs=wt["UwT"][:, 1, :], start=False, stop=False)
                nc.tensor.matmul(out=agg_ps[:], lhsT=ones_t[:1, :], rhs=ub_t[:1, :],
                                 start=False, stop=True)
                aout_t = io.tile([P, H], f32, name="aout_t", tag="aout_t")
                nc.scalar.activation(
                    out=aout_t[:], in_=agg_ps[:],
                    func=mybir.ActivationFunctionType.Relu,
                )
                nc.sync.dma_start(out=atomout_e[b * P:(b + 1) * P, :], in_=aout_t[:])

    nc.compile()
    return nc


def kernel(atom_feature_matrix, bond_feature_matrix, edge_src, edge_dst,
           Uw, Ub, Vw, Vb, Aw, Ab, Bw, Bb, Cw, Cb):
    x = np.ascontiguousarray(np.asarray(atom_feature_matrix, dtype=np.float32))
    bond = np.ascontiguousarray(np.asarray(bond_feature_matrix, dtype=np.float32))
    src = np.asarray(edge_src).astype(np.int64)
    dst = np.asarray(edge_dst).astype(np.int64)
    n_atoms, _ = x.shape
    n_edges = bond.shape[0]

    A_own = (n_atoms + NCORES - 1) // NCORES  # atoms per core (unpadded)
    NB = (A_own + P - 1) // P
    A_pc = NB * P

    perm = np.argsort(src, kind="stable")
    src_s = src[perm]
    dst_s = dst[perm]

    # per (core, block) edge ranges
    block_edges = []
    maxcnt = 1
    for c in range(NCORES):
        base = c * A_own
        rows = []
        for b in range(NB):
            lo = np.searchsorted(src_s, base + b * P, side="left")
            hi = np.searchsorted(src_s, min(base + (b + 1) * P, (c + 1) * A_own),
                                 side="left")
            rows.append((lo, hi))
            maxcnt = max(maxcnt, hi - lo)
        block_edges.append(rows)
    S = (maxcnt + P - 1) // P
    E_pc = NB * S * P

    nc = _PROG_CACHE.get((NB, S, n_atoms))
    if nc is None:
        nc = _build_program(NB, S, n_atoms)
        _PROG_CACHE[(NB, S, n_atoms)] = nc

    iott = np.tile(np.arange(P, dtype=np.float32), (P, 1))
    iotc = np.arange(P, dtype=np.float32).reshape(P, 1)
    common = {
        "xfull": x,
        "AwT": Aw.T.copy(), "BwT": Bw.T.copy(), "CwT": Cw.T.copy(),
        "VwT": Vw.T.copy(), "UwT": Uw.T.copy(),
        "bb": (Ab + Bb + Cb).reshape(1, H).astype(np.float32),
        "vb": Vb.reshape(1, H).astype(np.float32),
        "ub": Ub.reshape(1, H).astype(np.float32),
        "iott": iott, "iotc": iotc,
        "ones": np.ones((1, P), np.float32),
        "ident": np.eye(P, dtype=np.float32),
    }
    common = {k: np.ascontiguousarray(v, dtype=np.float32) for k, v in common.items()}

    in_maps = []
    metas = []
    for c in range(NCORES):
        base = c * A_own
        bond_pad = np.zeros((E_pc, H), np.float32)
        dsti = np.zeros((NB, P, S), np.int32)
        srcc = np.full((NB, P, S), -1.0, np.float32)
        atom_pad = np.zeros((A_pc, H), np.float32)
        n_own = min(A_own, n_atoms - base)
        atom_pad[:n_own] = x[base:base + n_own]
        sel_perm = np.zeros(E_pc, np.int64)
        sel_n = np.zeros(NB, np.int32)
        for b in range(NB):
            lo, hi = block_edges[c][b]
            n = hi - lo
            sel_n[b] = n
            if n == 0:
                continue
            r0 = b * S * P
            bond_pad[r0:r0 + n] = bond[perm[lo:hi]]
            sel_perm[r0:r0 + n] = perm[lo:hi]
            # edge position e within block -> (j, p) = (e // P, e % P)
            js, ps = np.divmod(np.arange(n), P)
            dsti[b, ps, js] = dst_s[lo:hi]
            srcc[b, ps, js] = (src_s[lo:hi] - (base + b * P)).astype(np.float32)
        srcr = np.ascontiguousarray(
            srcc.transpose(0, 2, 1).reshape(NB, S * P))  # [b, j*P + p]
        in_map = dict(common)
        in_map.update({
            "bond": bond_pad, "atom": atom_pad, "dsti": dsti,
            "srcc": srcc, "srcr": srcr,
        })
        in_maps.append(in_map)
        metas.append((sel_perm, sel_n))

    res = run_bass_kernel_spmd(nc, in_maps, list(range(NCORES)))

    atom_out = np.empty((n_atoms, H), np.float32)
    bond_out = np.empty((n_edges, H), np.float32)
    for c in range(NCORES):
        base = c * A_own
        n_own = min(A_own, n_atoms - base)
        atom_out[base:base + n_own] = res.results[c]["atomout"][:n_own]
        sel_perm, sel_n = metas[c]
        bo = res.results[c]["bondout"]
        for b in range(NB):
            n = int(sel_n[b])
            if n:
                r0 = b * S * P
                bond_out[sel_perm[r0:r0 + n]] = bo[r0:r0 + n]
    return atom_out, bond_out
